# revision 23
# baseline (speedup 1.0000x reference)
"""Trainium2 Bass kernel: causal multi-head attention block (B=2, S=2048, D=2048, H=16).

Sharding: 8 cores = 2 (batch) x 4 (head-groups of 4 heads). Each core computes
its batch's attention output restricted to its 4 heads plus the corresponding
partial out-projection; the host sums the 4 head-group partials per batch and
adds the (o_b + o_w @ v_b) bias vector (valid because softmax rows sum to 1).
The k-bias is dropped entirely: softmax over keys is invariant to per-query
logit shifts, so only (q + bq) . k survives.

v2: all matmuls in bf16 (full PE rate, half DMA/SBUF of fp32r), x tiles cached
in SBUF across the Q/K and V projection passes (x read from HBM once), DMA
issue interleaved per k-slice so the first matmul starts ~2us in, softmax
denominators accumulated on DVE + gpsimd partition_all_reduce (no PE row-sum
matmuls), phase-B software pipelining (score i+1 issued before attnV i), and
out-projection matmuls of block J-1 interleaved into phase B of block J.
"""

import sys

sys.path.insert(0, "/opt/trn_rl_repo")

import numpy as np
import ml_dtypes
import concourse.bacc as bacc
import concourse.tile as tile
from concourse import mybir
from concourse import bass_isa
from concourse.bass_utils import run_bass_kernel_spmd

F32 = mybir.dt.float32
BF16 = mybir.dt.bfloat16
BFNP = ml_dtypes.bfloat16

B, S, D, H, HD = 2, 2048, 2048, 16, 128
SCALE = 1.0 / (HD**0.5)
HL = 4  # heads per core
DL = HL * HD  # 512: local head dims per core
NK = D // HD  # 16 contraction k-tiles
NJ = S // DL  # 4 blocks of 512 along sequence
NEG = -1.0e30

_CACHE = {}


def _build():
    nc = bacc.Bacc("TRN2", target_bir_lowering=False, debug=False)
    ExpF = mybir.ActivationFunctionType.Exp
    IdF = mybir.ActivationFunctionType.Identity

    xt_d = nc.declare_dram_parameter("xt", [NJ * NK, HD, DL], BF16, isOutput=False)
    wq_d = nc.declare_dram_parameter("wq", [NK, HD, DL], BF16, isOutput=False)
    wk_d = nc.declare_dram_parameter("wk", [NK, HD, DL], BF16, isOutput=False)
    wv_d = nc.declare_dram_parameter("wv", [NK, HD, DL], BF16, isOutput=False)
    wo_d = nc.declare_dram_parameter("wo", [HL, HD, D], BF16, isOutput=False)
    bq_d = nc.declare_dram_parameter("bq", [HD, HL], F32, isOutput=False)
    mask_d = nc.declare_dram_parameter("maskT", [HD, HD], F32, isOutput=False)
    out_d = nc.declare_dram_parameter("out", [S, D], BF16, isOutput=True)

    with tile.TileContext(nc) as tc:
        with (
            tc.tile_pool(name="const", bufs=1) as constp,
            tc.tile_pool(name="wts", bufs=1) as wts,
            tc.tile_pool(name="qk", bufs=1) as qkp,
            tc.tile_pool(name="vres", bufs=1) as vp,
            tc.tile_pool(name="xc", bufs=2) as xcp,
            tc.tile_pool(name="ptt", bufs=6) as pttp,
            tc.tile_pool(name="pacc", bufs=2) as paccp,
            tc.tile_pool(name="den", bufs=2) as denp,
            tc.tile_pool(name="rec", bufs=2) as recp,
            tc.tile_pool(name="attn", bufs=2) as attnp,
            tc.tile_pool(name="ob", bufs=3) as obp,
        ):
            # --- weights (per-k slices, interleaved with x tiles of block 0
            # so the first projection matmul can start almost immediately) ---
            wq_sb = wts.tile([HD, NK * DL], BF16, tag="wq")
            wk_sb = wts.tile([HD, NK * DL], BF16, tag="wk")
            wv_sb = wts.tile([HD, NK * DL], BF16, tag="wv")
            bq_sb = constp.tile([HD, HL], F32, tag="bq")
            mask_sb = constp.tile([HD, HD], F32, tag="mask")

            # --- residents ---
            QT = [qkp.tile([HD, S], BF16, tag=f"qt{h}", name=f"qt{h}") for h in range(HL)]
            KT = [qkp.tile([HD, S], BF16, tag=f"kt{h}", name=f"kt{h}") for h in range(HL)]
            V = [vp.tile([HD, DL], BF16, tag=f"v{t}", name=f"v{t}") for t in range(S // HD)]

            def load_x_tile(J, k):
                xt = xcp.tile([HD, DL], BF16, tag=f"x{k}", name=f"x{J}_{k}")
                nc.sync.dma_start(xt[:], xt_d[J * NK + k])
                return xt

            def load_x_block(J):
                return [load_x_tile(J, k) for k in range(NK)]

            # triplets (wq_k, wk_k, x_k) in consumption order: the k=0
            # matmuls start after three 128KB transfers
            first_x = []
            for k in range(NK):
                sl_w = slice(DL * k, DL * (k + 1))
                nc.sync.dma_start(wq_sb[:, sl_w], wq_d[k])
                nc.sync.dma_start(wk_sb[:, sl_w], wk_d[k])
                first_x.append(load_x_tile(0, k))
                if k == 2:
                    nc.sync.dma_start(bq_sb[:], bq_d[:, :])
                    nc.sync.dma_start(mask_sb[:], mask_d[:, :])
            for k in range(NK):
                sl_w = slice(DL * k, DL * (k + 1))
                nc.sync.dma_start(wv_sb[:, sl_w], wv_d[k])

            # ================= PHASE A: projections =================
            x_blocks = [None] * NJ
            x_blocks[0] = first_x
            psA_ctx = tc.tile_pool(name="psA", bufs=8, space="PSUM")
            psA = psA_ctx.__enter__()
            for J in range(NJ):
                sl_s = slice(DL * J, DL * (J + 1))
                xts = x_blocks[J]
                # Q/K pass: 8 psum accumulators over the k loop
                qps = [psA.tile([HD, DL], F32, tag="ps", name=f"qps{J}_{h}") for h in range(HL)]
                kps = [psA.tile([HD, DL], F32, tag="ps", name=f"kps{J}_{h}") for h in range(HL)]
                for k in range(NK):
                    for h in range(HL):
                        sl_wh = slice(DL * k + HD * h, DL * k + HD * (h + 1))
                        nc.tensor.matmul(
                            qps[h][:], wq_sb[:, sl_wh], xts[k][:],
                            start=(k == 0), stop=(k == NK - 1),
                        )
                        nc.tensor.matmul(
                            kps[h][:], wk_sb[:, sl_wh], xts[k][:],
                            start=(k == 0), stop=(k == NK - 1),
                        )
                with nc.allow_low_precision(reason="bf16 QKV tiles"):
                    for h in range(HL):
                        nc.scalar.activation(
                            QT[h][:, sl_s], qps[h][:], IdF, bias=bq_sb[:, h : h + 1]
                        )
                    for h in range(HL):
                        nc.vector.tensor_copy(KT[h][:, sl_s], kps[h][:])
                # V pass: reuses the cached x tiles (no second HBM read)
                vps = [psA.tile([HD, DL], F32, tag="ps", name=f"vps{J}_{t}") for t in range(4)]
                for k in range(NK):
                    sl_wk = slice(DL * k, DL * (k + 1))
                    for t in range(4):
                        nc.tensor.matmul(
                            vps[t][:],
                            xts[k][:, HD * t : HD * (t + 1)],
                            wv_sb[:, sl_wk],
                            start=(k == 0), stop=(k == NK - 1),
                        )
                with nc.allow_low_precision(reason="bf16 V tiles"):
                    for t in range(4):
                        nc.scalar.copy(V[4 * J + t][:], vps[t][:])
                # prefetch next x block (double-buffered per-k tags)
                if J + 1 < NJ:
                    x_blocks[J + 1] = load_x_block(J + 1)

            psA_ctx.__exit__(None, None, None)

            # out-proj weights (needed from the first C chunk, ~40us into B)
            wo_sb = []
            for dh in range(HL):
                w = wts.tile([HD, D], BF16, tag=f"wo{dh}", name=f"wo{dh}")
                nc.sync.dma_start(w[:], wo_d[dh])
                wo_sb.append(w)

            # ============ PHASES B (attention) + C (out-proj) ============
            psB_ctx = tc.tile_pool(name="psB", bufs=1, space="PSUM")
            psB = psB_ctx.__enter__()
            attn_t = [[None] * HL for _ in range(NJ)]

            def c_gen(Jc):
                """Out-projection instruction stream for q-block Jc (yields
                after each instruction so it can interleave into phase B)."""
                at = attn_t[Jc]
                for c in range(4):
                    ob = obp.tile([HD, D], BF16, tag="ob", name=f"ob{Jc}_{c}")
                    sl_c = slice(HD * c, HD * (c + 1))
                    st = 4 * Jc + c
                    rows = slice(HD * st, HD * (st + 1))
                    tail = Jc == NJ - 1 and c >= 2
                    for nb in range(4):
                        sl_n = slice(DL * nb, DL * (nb + 1))
                        op = psB.tile([HD, DL], F32, tag="op", bufs=2, name=f"op{Jc}_{c}_{nb}")
                        for dh in range(HL):
                            nc.tensor.matmul(
                                op[:], at[dh][:, sl_c], wo_sb[dh][:, sl_n],
                                start=(dh == 0), stop=(dh == HL - 1),
                            )
                            yield
                        with nc.allow_low_precision(reason="bf16 out partials"):
                            # the drain tail runs after the last exp: Act is idle
                            if tail and nb % 2 == 0:
                                nc.scalar.copy(ob[:, sl_n], op[:])
                            else:
                                nc.vector.tensor_copy(ob[:, sl_n], op[:])
                        yield
                    nc.sync.dma_start(out_d[rows, :], ob[:])
                    yield

            _SENT = object()
            for J in range(NJ):
                cg = c_gen(J - 1) if J >= 1 else iter(())

                def pull(n):
                    for _ in range(n):
                        if next(cg, _SENT) is _SENT:
                            break

                nkt = 4 * (J + 1)
                for h in range(HL):
                    sl_h = slice(HD * h, HD * (h + 1))
                    aps = psB.tile([HD, DL], F32, tag="aps", bufs=3, name=f"aps{J}_{h}")
                    pacc = paccp.tile([HD, DL], BF16, tag="pacc")
                    pend = None
                    for i in range(nkt):
                        qlo = HD * (i - 4 * J) if i >= 4 * J else 0
                        cs = slice(qlo, DL)
                        qs = slice(DL * J + qlo, DL * (J + 1))
                        scp = psB.tile([HD, DL], F32, tag="scp", bufs=3, name=f"scp{J}_{h}_{i}")
                        nc.tensor.matmul(
                            scp[:, cs], KT[h][:, HD * i : HD * (i + 1)], QT[h][:, qs],
                            start=True, stop=True,
                        )
                        if pend is not None:
                            ip, csp, pttp_ = pend
                            nc.tensor.matmul(
                                aps[:, csp], V[ip][:, sl_h], pttp_[:, csp],
                                start=(ip == 0), stop=False,
                            )
                        pull(2)
                        if i >= 4 * J:
                            # causal mask only on the 128x128 diagonal square
                            dsl = slice(qlo, qlo + HD)
                            nc.vector.tensor_add(scp[:, dsl], scp[:, dsl], mask_sb[:])
                        ptt = pttp.tile([HD, DL], BF16, tag="pt")
                        with nc.allow_low_precision(reason="bf16 softmax probs"):
                            nc.scalar.activation(ptt[:, cs], scp[:, cs], ExpF)
                            if i == 0:
                                nc.vector.tensor_copy(pacc[:], ptt[:])
                            else:
                                nc.vector.tensor_add(pacc[:, cs], pacc[:, cs], ptt[:, cs])
                        pend = (i, cs, ptt)
                    ip, csp, pttp_ = pend
                    nc.tensor.matmul(
                        aps[:, csp], V[ip][:, sl_h], pttp_[:, csp],
                        start=(ip == 0), stop=True,
                    )
                    den = denp.tile([HD, DL], F32, tag="den")
                    nc.gpsimd.partition_all_reduce(den[:], pacc[:], 128, bass_isa.ReduceOp.add)
                    rec = recp.tile([HD, DL], F32, tag="rec")
                    nc.vector.reciprocal_approx_fast(rec[:], den[:])
                    at = attnp.tile([HD, DL], BF16, tag=f"at{h}", name=f"at{J}_{h}")
                    with nc.allow_low_precision(reason="bf16 attn tiles"):
                        nc.vector.tensor_mul(at[:], aps[:], rec[:])
                    attn_t[J][h] = at
                    pull(2)
                # drain the rest of C(J-1)
                for _ in cg:
                    pass
            # tail: C(3)
            for _ in c_gen(NJ - 1):
                pass
            psB_ctx.__exit__(None, None, None)

    nc.compile()
    return nc


def _prep_in_maps(x, q_w, q_b, k_w, k_b, v_w, v_b, o_w, o_b):
    mask = np.where(
        np.arange(HD)[:, None] > np.arange(HD)[None, :], np.float32(NEG), np.float32(0)
    ).astype(np.float32)
    in_maps = []
    for c in range(8):
        b, hg = divmod(c, 4)
        ds = slice(DL * hg, DL * (hg + 1))
        xT = np.ascontiguousarray(x[b].T.astype(BFNP))  # [D, S]
        xt = np.ascontiguousarray(
            xT.reshape(NK, HD, NJ, DL).transpose(2, 0, 1, 3).reshape(NJ * NK, HD, DL)
        )
        wq = np.ascontiguousarray((q_w[ds].T * SCALE).astype(BFNP).reshape(NK, HD, DL))
        wk = np.ascontiguousarray(k_w[ds].T.astype(BFNP).reshape(NK, HD, DL))
        wv = np.ascontiguousarray(v_w[ds].T.astype(BFNP).reshape(NK, HD, DL))
        wo = np.ascontiguousarray(o_w[:, ds].T.astype(BFNP).reshape(HL, HD, D))
        in_maps.append(
            {
                "xt": xt,
                "wq": wq,
                "wk": wk,
                "wv": wv,
                "wo": wo,
                "bq": np.ascontiguousarray((q_b[ds] * SCALE).reshape(HL, HD).T),
                "maskT": mask,
            }
        )
    return in_maps


def kernel(x, q_w, q_b, k_w, k_b, v_w, v_b, o_w, o_b, _trace=False, _trace_kwargs=None):
    x = np.asarray(x, np.float32)
    args = [np.asarray(a, np.float32) for a in (q_w, q_b, k_w, k_b, v_w, v_b, o_w, o_b)]
    q_w, q_b, k_w, k_b, v_w, v_b, o_w, o_b = args

    if "nc" not in _CACHE:
        _CACHE["nc"] = _build()
    nc = _CACHE["nc"]

    in_maps = _prep_in_maps(x, q_w, q_b, k_w, k_b, v_w, v_b, o_w, o_b)
    res = run_bass_kernel_spmd(
        nc, in_maps, list(range(8)), trace=_trace, **(_trace_kwargs or {})
    )
    _CACHE["last_result"] = res

    bias_vec = (o_w @ v_b + o_b).astype(np.float32)
    out = np.empty((B, S, D), np.float32)
    for b in range(B):
        acc = res.results[4 * b]["out"].astype(np.float32)
        for hg in range(1, 4):
            acc = acc + res.results[4 * b + hg]["out"].astype(np.float32)
        out[b] = acc + bias_vec
    return out


# revision 32
# speedup vs baseline: 1.0219x; 1.0219x over previous
"""Trainium2 Bass kernel: causal multi-head attention block (B=2, S=2048, D=2048, H=16).

Sharding: 8 cores = 2 (batch) x 4 (head-groups of 4 heads). Each core computes
its batch's attention output restricted to its 4 heads plus the corresponding
partial out-projection; the host sums the 4 head-group partials per batch and
adds the (o_b + o_w @ v_b) bias vector (valid because softmax rows sum to 1).
The k-bias is dropped entirely: softmax over keys is invariant to per-query
logit shifts, so only (q + bq) . k survives.

v2: all matmuls in bf16 (full PE rate, half DMA/SBUF of fp32r), x tiles cached
in SBUF across the Q/K and V projection passes (x read from HBM once), DMA
issue interleaved per k-slice so the first matmul starts ~2us in, softmax
denominators accumulated on DVE + gpsimd partition_all_reduce (no PE row-sum
matmuls), phase-B software pipelining (score i+1 issued before attnV i), and
out-projection matmuls of block J-1 interleaved into phase B of block J.
"""

import sys

sys.path.insert(0, "/opt/trn_rl_repo")

import numpy as np
import ml_dtypes
import concourse.bacc as bacc
import concourse.tile as tile
from concourse import mybir
from concourse import bass_isa
from concourse.bass_utils import run_bass_kernel_spmd

F32 = mybir.dt.float32
BF16 = mybir.dt.bfloat16
BFNP = ml_dtypes.bfloat16

B, S, D, H, HD = 2, 2048, 2048, 16, 128
SCALE = 1.0 / (HD**0.5)
HL = 4  # heads per core
DL = HL * HD  # 512: local head dims per core
NK = D // HD  # 16 contraction k-tiles
NJ = S // DL  # 4 blocks of 512 along sequence
NEG = -1.0e30

_CACHE = {}


def _build():
    nc = bacc.Bacc("TRN2", target_bir_lowering=False, debug=False)
    ExpF = mybir.ActivationFunctionType.Exp
    IdF = mybir.ActivationFunctionType.Identity

    xt_d = nc.declare_dram_parameter("xt", [NJ * NK, HD, DL], BF16, isOutput=False)
    wq_d = nc.declare_dram_parameter("wq", [NK, HD, DL], BF16, isOutput=False)
    wk_d = nc.declare_dram_parameter("wk", [NK, HD, DL], BF16, isOutput=False)
    wv_d = nc.declare_dram_parameter("wv", [NK, HD, DL], BF16, isOutput=False)
    wo_d = nc.declare_dram_parameter("wo", [HL, HD, D], BF16, isOutput=False)
    bq_d = nc.declare_dram_parameter("bq", [HD, HL], F32, isOutput=False)
    mask_d = nc.declare_dram_parameter("maskT", [HD, HD], F32, isOutput=False)
    out_d = nc.declare_dram_parameter("out", [S, D], BF16, isOutput=True)

    with tile.TileContext(nc) as tc:
        with (
            tc.tile_pool(name="const", bufs=1) as constp,
            tc.tile_pool(name="wts", bufs=1) as wts,
            tc.tile_pool(name="qk", bufs=1) as qkp,
            tc.tile_pool(name="vres", bufs=1) as vp,
            tc.tile_pool(name="xc", bufs=2) as xcp,
            tc.tile_pool(name="ptt", bufs=6) as pttp,
            tc.tile_pool(name="pacc", bufs=2) as paccp,
            tc.tile_pool(name="den", bufs=2) as denp,
            tc.tile_pool(name="rec", bufs=2) as recp,
            tc.tile_pool(name="attn", bufs=2) as attnp,
            tc.tile_pool(name="ob", bufs=3) as obp,
        ):
            # --- weights (per-k slices, interleaved with x tiles of block 0
            # so the first projection matmul can start almost immediately) ---
            wq_sb = wts.tile([HD, NK * DL], BF16, tag="wq")
            wk_sb = wts.tile([HD, NK * DL], BF16, tag="wk")
            wv_sb = wts.tile([HD, NK * DL], BF16, tag="wv")
            bq_sb = constp.tile([HD, HL], F32, tag="bq")
            mask_sb = constp.tile([HD, HD], F32, tag="mask")

            # --- residents ---
            QT = [qkp.tile([HD, S], BF16, tag=f"qt{h}", name=f"qt{h}") for h in range(HL)]
            KT = [qkp.tile([HD, S], BF16, tag=f"kt{h}", name=f"kt{h}") for h in range(HL)]
            V = [vp.tile([HD, DL], BF16, tag=f"v{t}", name=f"v{t}") for t in range(S // HD)]

            def load_x_tile(J, k):
                xt = xcp.tile([HD, DL], BF16, tag=f"x{k}", name=f"x{J}_{k}")
                nc.sync.dma_start(xt[:], xt_d[J * NK + k])
                return xt

            def load_x_block(J):
                return [load_x_tile(J, k) for k in range(NK)]

            # triplets (wq_k, wk_k, x_k) in consumption order: the k=0
            # matmuls start after three 128KB transfers
            first_x = []
            for k in range(NK):
                sl_w = slice(DL * k, DL * (k + 1))
                nc.sync.dma_start(wq_sb[:, sl_w], wq_d[k])
                nc.sync.dma_start(wk_sb[:, sl_w], wk_d[k])
                first_x.append(load_x_tile(0, k))
                if k == 2:
                    nc.sync.dma_start(bq_sb[:], bq_d[:, :])
                    nc.sync.dma_start(mask_sb[:], mask_d[:, :])
            for k in range(NK):
                sl_w = slice(DL * k, DL * (k + 1))
                nc.sync.dma_start(wv_sb[:, sl_w], wv_d[k])

            # ================= PHASE A: projections =================
            x_blocks = [None] * NJ
            x_blocks[0] = first_x
            psA_ctx = tc.tile_pool(name="psA", bufs=8, space="PSUM")
            psA = psA_ctx.__enter__()
            for J in range(NJ):
                sl_s = slice(DL * J, DL * (J + 1))
                xts = x_blocks[J]
                # Q/K pass: 8 psum accumulators over the k loop
                qps = [psA.tile([HD, DL], F32, tag="ps", name=f"qps{J}_{h}") for h in range(HL)]
                kps = [psA.tile([HD, DL], F32, tag="ps", name=f"kps{J}_{h}") for h in range(HL)]
                for k in range(NK):
                    for h in range(HL):
                        sl_wh = slice(DL * k + HD * h, DL * k + HD * (h + 1))
                        nc.tensor.matmul(
                            qps[h][:], wq_sb[:, sl_wh], xts[k][:],
                            start=(k == 0), stop=(k == NK - 1),
                        )
                        nc.tensor.matmul(
                            kps[h][:], wk_sb[:, sl_wh], xts[k][:],
                            start=(k == 0), stop=(k == NK - 1),
                        )
                with nc.allow_low_precision(reason="bf16 QKV tiles"):
                    for h in range(HL):
                        nc.scalar.activation(
                            QT[h][:, sl_s], qps[h][:], IdF, bias=bq_sb[:, h : h + 1]
                        )
                    for h in range(HL):
                        nc.any.tensor_copy(KT[h][:, sl_s], kps[h][:])
                # V pass: reuses the cached x tiles (no second HBM read)
                vps = [psA.tile([HD, DL], F32, tag="ps", name=f"vps{J}_{t}") for t in range(4)]
                for k in range(NK):
                    sl_wk = slice(DL * k, DL * (k + 1))
                    for t in range(4):
                        nc.tensor.matmul(
                            vps[t][:],
                            xts[k][:, HD * t : HD * (t + 1)],
                            wv_sb[:, sl_wk],
                            start=(k == 0), stop=(k == NK - 1),
                        )
                with nc.allow_low_precision(reason="bf16 V tiles"):
                    for t in range(4):
                        nc.scalar.copy(V[4 * J + t][:], vps[t][:])
                # prefetch next x block (double-buffered per-k tags)
                if J + 1 < NJ:
                    x_blocks[J + 1] = load_x_block(J + 1)

            psA_ctx.__exit__(None, None, None)

            # out-proj weights (needed from the first C chunk, ~40us into B)
            wo_sb = []
            for dh in range(HL):
                w = wts.tile([HD, D], BF16, tag=f"wo{dh}", name=f"wo{dh}")
                nc.sync.dma_start(w[:], wo_d[dh])
                wo_sb.append(w)

            # ============ PHASES B (attention) + C (out-proj) ============
            psB_ctx = tc.tile_pool(name="psB", bufs=1, space="PSUM")
            psB = psB_ctx.__enter__()
            attn_t = [[None] * HL for _ in range(NJ)]

            def c_gen(Jc):
                """Out-projection instruction stream for q-block Jc (yields
                after each instruction so it can interleave into phase B)."""
                at = attn_t[Jc]
                for c in range(4):
                    ob = obp.tile([HD, D], BF16, tag="ob", name=f"ob{Jc}_{c}")
                    sl_c = slice(HD * c, HD * (c + 1))
                    st = 4 * Jc + c
                    rows = slice(HD * st, HD * (st + 1))
                    for nb in range(4):
                        sl_n = slice(DL * nb, DL * (nb + 1))
                        op = psB.tile([HD, DL], F32, tag="op", bufs=3, name=f"op{Jc}_{c}_{nb}")
                        for dh in range(HL):
                            nc.tensor.matmul(
                                op[:], at[dh][:, sl_c], wo_sb[dh][:, sl_n],
                                start=(dh == 0), stop=(dh == HL - 1),
                            )
                            yield
                        with nc.allow_low_precision(reason="bf16 out partials"):
                            nc.any.tensor_copy(ob[:, sl_n], op[:])
                        yield
                    nc.sync.dma_start(out_d[rows, :], ob[:])
                    yield

            _SENT = object()
            for J in range(NJ):
                cg = c_gen(J - 1) if J >= 1 else iter(())

                def pull(n):
                    for _ in range(n):
                        if next(cg, _SENT) is _SENT:
                            break

                nkt = 4 * (J + 1)
                for h in range(HL):
                    sl_h = slice(HD * h, HD * (h + 1))
                    aps = psB.tile([HD, DL], F32, tag="aps", bufs=2, name=f"aps{J}_{h}")
                    pacc = paccp.tile([HD, DL], BF16, tag="pacc")
                    pend = None
                    for i in range(nkt):
                        qlo = HD * (i - 4 * J) if i >= 4 * J else 0
                        cs = slice(qlo, DL)
                        qs = slice(DL * J + qlo, DL * (J + 1))
                        scp = psB.tile([HD, DL], F32, tag="scp", bufs=3, name=f"scp{J}_{h}_{i}")
                        nc.tensor.matmul(
                            scp[:, cs], KT[h][:, HD * i : HD * (i + 1)], QT[h][:, qs],
                            start=True, stop=True,
                        )
                        if pend is not None:
                            ip, csp, pttp_ = pend
                            nc.tensor.matmul(
                                aps[:, csp], V[ip][:, sl_h], pttp_[:, csp],
                                start=(ip == 0), stop=False,
                            )
                        pull(2)
                        if i >= 4 * J:
                            # causal mask only on the 128x128 diagonal square
                            dsl = slice(qlo, qlo + HD)
                            nc.any.tensor_add(scp[:, dsl], scp[:, dsl], mask_sb[:])
                        ptt = pttp.tile([HD, DL], BF16, tag="pt")
                        with nc.allow_low_precision(reason="bf16 softmax probs"):
                            nc.scalar.activation(ptt[:, cs], scp[:, cs], ExpF)
                            if i == 0:
                                nc.any.tensor_copy(pacc[:], ptt[:])
                            else:
                                nc.any.tensor_add(pacc[:, cs], pacc[:, cs], ptt[:, cs])
                        pend = (i, cs, ptt)
                    ip, csp, pttp_ = pend
                    nc.tensor.matmul(
                        aps[:, csp], V[ip][:, sl_h], pttp_[:, csp],
                        start=(ip == 0), stop=True,
                    )
                    den = denp.tile([HD, DL], F32, tag="den")
                    nc.gpsimd.partition_all_reduce(den[:], pacc[:], 128, bass_isa.ReduceOp.add)
                    rec = recp.tile([HD, DL], F32, tag="rec")
                    nc.vector.reciprocal_approx_fast(rec[:], den[:])
                    at = attnp.tile([HD, DL], BF16, tag=f"at{h}", name=f"at{J}_{h}")
                    with nc.allow_low_precision(reason="bf16 attn tiles"):
                        nc.any.tensor_mul(at[:], aps[:], rec[:])
                    attn_t[J][h] = at
                    pull(2)
                # drain the rest of C(J-1)
                for _ in cg:
                    pass
            # tail: C(3)
            for _ in c_gen(NJ - 1):
                pass
            psB_ctx.__exit__(None, None, None)

    nc.compile()
    return nc


def _prep_in_maps(x, q_w, q_b, k_w, k_b, v_w, v_b, o_w, o_b):
    mask = np.where(
        np.arange(HD)[:, None] > np.arange(HD)[None, :], np.float32(NEG), np.float32(0)
    ).astype(np.float32)
    in_maps = []
    for c in range(8):
        b, hg = divmod(c, 4)
        ds = slice(DL * hg, DL * (hg + 1))
        xT = np.ascontiguousarray(x[b].T.astype(BFNP))  # [D, S]
        xt = np.ascontiguousarray(
            xT.reshape(NK, HD, NJ, DL).transpose(2, 0, 1, 3).reshape(NJ * NK, HD, DL)
        )
        wq = np.ascontiguousarray((q_w[ds].T * SCALE).astype(BFNP).reshape(NK, HD, DL))
        wk = np.ascontiguousarray(k_w[ds].T.astype(BFNP).reshape(NK, HD, DL))
        wv = np.ascontiguousarray(v_w[ds].T.astype(BFNP).reshape(NK, HD, DL))
        wo = np.ascontiguousarray(o_w[:, ds].T.astype(BFNP).reshape(HL, HD, D))
        in_maps.append(
            {
                "xt": xt,
                "wq": wq,
                "wk": wk,
                "wv": wv,
                "wo": wo,
                "bq": np.ascontiguousarray((q_b[ds] * SCALE).reshape(HL, HD).T),
                "maskT": mask,
            }
        )
    return in_maps


def kernel(x, q_w, q_b, k_w, k_b, v_w, v_b, o_w, o_b, _trace=False, _trace_kwargs=None):
    x = np.asarray(x, np.float32)
    args = [np.asarray(a, np.float32) for a in (q_w, q_b, k_w, k_b, v_w, v_b, o_w, o_b)]
    q_w, q_b, k_w, k_b, v_w, v_b, o_w, o_b = args

    if "nc" not in _CACHE:
        _CACHE["nc"] = _build()
    nc = _CACHE["nc"]

    in_maps = _prep_in_maps(x, q_w, q_b, k_w, k_b, v_w, v_b, o_w, o_b)
    res = run_bass_kernel_spmd(
        nc, in_maps, list(range(8)), trace=_trace, **(_trace_kwargs or {})
    )
    _CACHE["last_result"] = res

    bias_vec = (o_w @ v_b + o_b).astype(np.float32)
    out = np.empty((B, S, D), np.float32)
    for b in range(B):
        acc = res.results[4 * b]["out"].astype(np.float32)
        for hg in range(1, 4):
            acc = acc + res.results[4 * b + hg]["out"].astype(np.float32)
        out[b] = acc + bias_vec
    return out
